# revision 5
# baseline (speedup 1.0000x reference)
"""Trainium2 Bass kernel for nn_CriticOld (twin-Q GNN critic: per-sample kNN +
EdgeConv + MLP head), data-parallel over batch across 8 NeuronCores.

V3 layout (per core): 128 problems = 2 Q-nets x 64 samples; col = p*32 + i
(i<30 real, 2 dead cols per problem), COLS = 4096. EdgeConv pairs are built
ON THE PE as psum1 = V_sb.T @ R_V + U_sb.T @ R_U where R_V is a runtime
one-hot (built by a 4x-packed DVE is_equal against broadcast index codes)
and R_U a constant one-hot. This kills the HBM V-table, the 61k-descriptor
dma_gather and the identity-recombine matmuls of the previous version.
kNN distances run on the Pool engine, top-15 on DVE, relu-evac on Act,
slot-max on DVE over dual-bank psum tiles.
"""
import sys

sys.path.insert(0, "/opt/trn_rl_repo")

import numpy as np
import ml_dtypes

import concourse.bass as bass
from concourse import bacc
import concourse.mybir as mybir
import concourse.tile as tile
from concourse import library_config
from concourse.bass_utils import run_bass_kernel_spmd
from concourse.vector_clock import ScopedClock

f32 = mybir.dt.float32
f32r = mybir.dt.float32r
bf16 = mybir.dt.bfloat16
u16 = mybir.dt.uint16
i16 = mybir.dt.int16
AF = mybir.ActivationFunctionType
OP = mybir.AluOpType
AX = mybir.AxisListType

BS, N, K, HID, EMB = 512, 30, 15, 128, 64
CORES = 8
BSC = BS // CORES          # samples per core
P = 2 * BSC                # 128 problems per core (2 Q-nets)
ST = 32                    # col stride per problem (30 real + 2 dead)
COLS = P * ST              # 4096
NG = 64                    # phase-D groups (2 problems each)
GC = 2 * K * N             # 900 pair cols per group
HC_ = K * N                # 450 pair cols per problem

import os
KNOB_POOL_CMP = int(os.environ.get("KPOOLCMP", "0"))  # every n-th group's compare on Pool
KNOB_POOL_DIST = os.environ.get("KPOOLDIST", "0") == "1"


def _patched_drain_and_barrier(self, tick_clock, wait_clock):
    # this walrus build caps sync-waits at 1/instruction; spread the
    # end-of-kernel waits over SP NOPs instead of one multi-wait Drain.
    nc = self.nc
    probe = nc.sync.nop()
    wait_clock.add_sem_waits(probe.ins, ScopedClock({None: tick_clock.global_clock}))
    si = probe.ins.sync_info
    waits = list(si.on_wait) if si is not None else []
    if len(waits) > 1:
        si.on_wait = [waits[0]]
        for w in waits[1:]:
            extra = nc.sync.nop()
            esi = extra.ins.sync_info
            if esi is None:
                extra.ins.sync_info = mybir.SyncInfo(on_wait=[w], on_update=[])
            else:
                esi.on_wait = [w]
    nc.sync.drain()
    nc.all_engine_barrier()
    assert self.sems is not None
    popped = nc._tile_sem_poison_stack.pop()
    assert popped is self._sem_poison
    nc.clear_and_free_semaphores(list(self.sems.allocated().values()))
    nc.all_engine_barrier()


tile.TileContext._drain_and_barrier = _patched_drain_and_barrier

_ws_cnt = [0]


def split_excess_waits(nc):
    """This walrus build supports at most 1 sync-wait per instruction (2 for
    EventSemaphore). Tile attaches several; move the extras onto same-engine
    NOPs inserted right before the instruction."""
    for fn in nc.m.functions:
        for bb in fn.blocks:
            new_list = []
            for inst in bb.instructions:
                si = inst.sync_info
                cap = 2 if isinstance(inst, mybir.InstEventSemaphore) else 1
                if si is not None and si.on_wait is not None and len(si.on_wait) > cap:
                    waits = list(si.on_wait)
                    for w in waits[:-cap]:
                        n = mybir.InstNoOp(name=f"I-wsplit-{_ws_cnt[0]}", ins=[], outs=[])
                        _ws_cnt[0] += 1
                        n.engine = inst.engine
                        n.sync_info = mybir.SyncInfo(on_wait=[w], on_update=[])
                        nc.register_instruction(n, overwrite=True)
                        new_list.append(n)
                    si.on_wait = waits[-cap:]
                new_list.append(inst)
            bb.instructions[:] = new_list


def ap_of(t, offset, dims):
    """Build a raw AP on tile/dram tensor t: dims = [[step, count], ...] (elements)."""
    base = t.ap() if hasattr(t, "ap") and not isinstance(t, bass.AP) else t
    return bass.AP(tensor=base.tensor, offset=base.offset + offset, ap=dims)


def build_program():
    nc = bacc.Bacc("TRN2", debug=False)

    din = {}
    def inp(name, shape, dtype=f32):
        din[name] = nc.dram_tensor(name, shape, dtype, kind="ExternalInput")
        return din[name]

    inp("xst", [16, COLS], bf16)          # [x_hi; x_lo; x_hi; x_lo]
    inp("x_pp", [P, N * 4])
    inp("wiT1d", [16, HID], bf16); inp("wiT2d", [16, HID], bf16)
    inp("wa2aT_hi", [HID, HID], bf16); inp("wa2aT_lo", [HID, HID], bf16)
    inp("wauT_hi", [HID, HID], bf16); inp("wauT_lo", [HID, HID], bf16)
    inp("c6v", [6, HID], bf16); inp("c6u", [6, HID], bf16)
    inp("ind6", [6, COLS], bf16)
    inp("wm1bT", [HID, HID], f32)
    inp("wcaT1", [HID, HID]); inp("wcaT2", [HID, HID])
    inp("wcb1", [HID, 1]); inp("wcb2", [HID, 1])
    inp("bi1", [HID, 1]); inp("bi2", [HID, 1])
    inp("bm1a", [HID, 1]); inp("bm1b", [HID, 1])
    inp("bca1", [HID, 1]); inp("bca2", [HID, 1])
    inp("r_u", [64, GC], bf16)
    inp("iota64", [64, 1])
    inp("p32", [P, 1])

    idxc_dram = nc.dram_tensor("idxc_dram", [P, HC_], i16)  # internal
    qout = nc.dram_tensor("qout", [1, P * N], f32, kind="ExternalOutput")

    from contextlib import ExitStack
    ctx = ExitStack()
    with tile.TileContext(nc) as tc, ctx:
        consts = ctx.enter_context(tc.tile_pool(name="consts", bufs=1))
        big = ctx.enter_context(tc.tile_pool(name="big", bufs=1))
        knn = ctx.enter_context(tc.tile_pool(name="knn", bufs=1))
        t8p = ctx.enter_context(tc.tile_pool(name="t8p", bufs=8))
        bcp = ctx.enter_context(tc.tile_pool(name="bcp", bufs=2))
        rpool = ctx.enter_context(tc.tile_pool(name="rpool", bufs=1))
        hpool = ctx.enter_context(tc.tile_pool(name="hpool", bufs=4))
        ps_1 = ctx.enter_context(tc.tile_pool(name="ps_1", bufs=2, space="PSUM"))
        ps_2 = ctx.enter_context(tc.tile_pool(name="ps_2", bufs=1, space="PSUM"))

        if KNOB_POOL_DIST or KNOB_POOL_CMP:
            nc.gpsimd.load_library(library_config.mlp)

        # ---- load inputs to SBUF ----
        sb = {}
        F32R_NAMES = ("wm1bT", "wcaT1", "wcaT2", "wcb1", "wcb2")
        for name in din:
            t = din[name]
            dt = f32r if name in F32R_NAMES else t.dtype
            st_ = consts.tile([t.shape[0], t.shape[1]], dt, tag=name)
            src_ap = t.ap().bitcast(f32r) if name in F32R_NAMES else t.ap()
            nc.sync.dma_start(out=st_[:], in_=src_ap)
            sb[name] = st_

        # ---- kNN distances (Pool) ----
        xpp = sb["x_pp"]
        diff = knn.tile([P, 3600], f32)
        sqd = knn.tile([P, 3600], f32)
        negd = knn.tile([P, 900], f32)
        nc.vector.tensor_tensor(
            out=diff[:],
            in0=ap_of(xpp, 0, [list(xpp.ap[0]), [4, N], [0, N], [1, 4]]),
            in1=ap_of(xpp, 0, [list(xpp.ap[0]), [0, N], [4, N], [1, 4]]),
            op=OP.subtract)
        nc.scalar.activation(sqd[:], diff[:], AF.Square)
        # negd = -sum(diff^2); self-distance 0 is always rank 1 of the top-k,
        # so the diagonal needs no exclusion mask (skipped positionally below)
        nc.vector.tensor_reduce(
            out=negd[:], in_=ap_of(sqd, 0, [list(sqd.ap[0]), [4, 900], [1, 4]]),
            axis=AX.X, op=OP.add, negate=True)

        # ---- top-15 per node (DVE) ----
        idxall = knn.tile([P, 16 * N], u16)
        for i in range(N):
            nd = negd[:, i * 30:(i + 1) * 30]
            m8 = t8p.tile([P, 8], f32, tag="m8")
            scr = t8p.tile([P, 30], f32, tag="scr")
            m8b = t8p.tile([P, 8], f32, tag="m8b")
            nc.vector.max(m8[:], nd)
            nc.vector.max_index(idxall[:, i * 16:i * 16 + 8], m8[:], nd)
            nc.vector.match_replace(scr[:], in_to_replace=m8[:], in_values=nd,
                                    imm_value=-1e30)
            nc.vector.max(m8b[:], scr[:])
            nc.vector.max_index(idxall[:, i * 16 + 8:i * 16 + 16], m8b[:], scr[:])

        # idxc_P = idxall (c-order i*15+s) + 32*(p%2), as i16 in DRAM
        gfp = knn.tile([P, HC_], f32)
        # slot 0 of round-1 max8 is the self-match (distance 0): skip it
        nc.vector.tensor_scalar(
            out=ap_of(gfp, 0, [list(gfp.ap[0]), [15, N], [1, 7]]),
            in0=ap_of(idxall, 1, [list(idxall.ap[0]), [16, N], [1, 7]]),
            scalar1=sb["p32"][:], scalar2=None, op0=OP.add)
        nc.vector.tensor_scalar(
            out=ap_of(gfp, 7, [list(gfp.ap[0]), [15, N], [1, 8]]),
            in0=ap_of(idxall, 8, [list(idxall.ap[0]), [16, N], [1, 8]]),
            scalar1=sb["p32"][:], scalar2=None, op0=OP.add)
        gi16 = knn.tile([P, HC_], i16)
        nc.vector.tensor_copy(gi16[:], gfp[:])
        nc.sync.dma_start(out=idxc_dram.ap(), in_=gi16[:])

        # broadcast index codes: 8 chunks x [64 part, 8 groups * 900] i16
        bch = []
        for c in range(8):
            t = bcp.tile([64, 8 * GC], i16, tag="bc", name=f"bc_{c}")
            src = bass.AP(tensor=idxc_dram, offset=(16 * c) * HC_,
                          ap=[[0, 64], [2 * HC_, 8], [HC_, 2], [1, HC_]])
            nc.sync.dma_start(out=t[:], in_=src)
            bch.append(t)

        # ---- phase A: init1 = relu(x W_i^T + b_i), bf16 feature-major ----
        init1 = big.tile([HID, COLS], bf16, tag="bigA")
        for t in range(8):
            q = t // 4
            sl = slice(t * 512, (t + 1) * 512)
            ps = ps_1.tile([HID, 1024], f32, tag="ps1", name=f"psA_{t}")
            nc.tensor.matmul(ps[:, 0:512], lhsT=sb["wiT1d" if q == 0 else "wiT2d"][:],
                             rhs=sb["xst"][:, sl], start=True, stop=True)
            nc.scalar.activation(init1[:, sl], ps[:, 0:512], AF.Relu,
                                 bias=sb["bi1" if q == 0 else "bi2"][:])

        # ---- VU_sb: node-major values, V at partitions 0:64, U at 64:128 ----
        # VU_sb[:, blk*128+f]: partition p<64 = V[col=blk*64+p, f],
        #                      partition 64+p = U[col=blk*64+p, f].
        VU_sb = big.tile([HID, 64 * HID], bf16)
        for t8 in range(8):  # 8 x 64-col blocks per psum tile
            ps = ps_1.tile([HID, 1024], f32, tag="ps1", name=f"psVU_{t8}")
            for k in range(8):
                blk = t8 * 8 + k
                lhs = init1[:, blk * 64:(blk + 1) * 64]
                ind = sb["ind6"][:, blk * 64:(blk + 1) * 64]
                seg = slice(k * 128, k * 128 + 128)
                nc.tensor.matmul(ps[0:64, seg], lhsT=lhs, rhs=sb["wa2aT_hi"][:],
                                 start=True, stop=False)
                nc.tensor.matmul(ps[0:64, seg], lhsT=lhs, rhs=sb["wa2aT_lo"][:],
                                 start=False, stop=False)
                nc.tensor.matmul(ps[0:64, seg], lhsT=ind, rhs=sb["c6v"][:],
                                 start=False, stop=True)
                nc.tensor.matmul(ps[64:128, seg], lhsT=lhs, rhs=sb["wauT_hi"][:],
                                 start=True, stop=False)
                nc.tensor.matmul(ps[64:128, seg], lhsT=lhs, rhs=sb["wauT_lo"][:],
                                 start=False, stop=False)
                nc.tensor.matmul(ps[64:128, seg], lhsT=ind, rhs=sb["c6u"][:],
                                 start=False, stop=True)
            nc.scalar.activation(VU_sb[:, t8 * 1024:(t8 + 1) * 1024], ps[:],
                                 AF.Copy)

        # ---- phase D: 64 groups of 2 problems / 900 pair-cols ----
        # R buffers: rows 64:128 hold the constant U one-hots (loaded once
        # straight from the r_u input); compare rewrites rows 0:64 per group.
        NRV = 6
        rvb = []
        for i in range(NRV):
            t = rpool.tile([HID, GC], bf16, tag=f"rv{i}")
            nc.sync.dma_start(out=t[64:128, :], in_=din["r_u"].ap())
            rvb.append(t)
        Hfin = big.tile([HID, P * N], f32)
        ps2big = ps_2.tile([HID, 2048], f32, tag="ps2")
        for g in range(NG):
            rv = rvb[g % NRV]
            nc.vector.tensor_scalar(out=rv[0:64, :],
                               in0=bch[g // 8][:, (g % 8) * GC:(g % 8 + 1) * GC],
                               scalar1=sb["iota64"][:], scalar2=None,
                               op0=OP.is_equal)
            ps1 = ps_1.tile([HID, 1024], f32, tag="ps1", name=f"ps1_{g}")
            for pm in range(2):
                nc.tensor.matmul(ps1[:, pm * 512:pm * 512 + HC_],
                                 lhsT=VU_sb[:, g * 128:(g + 1) * 128],
                                 rhs=rv[:, pm * HC_:(pm + 1) * HC_],
                                 start=True, stop=True)
            h = hpool.tile([HID, GC], f32r, tag="h")
            nc.scalar.activation(
                ap_of(h, 0, [list(h.ap[0]), [HC_, 2], [1, HC_]]),
                ap_of(ps1, 0, [list(ps1.ap[0]), [512, 2], [1, HC_]]),
                AF.Relu, bias=sb["bm1a"][:])
            po = (g % 2) * 1024
            for pm in range(2):
                nc.tensor.matmul(ps2big[:, po + pm * 512:po + pm * 512 + HC_],
                                 lhsT=sb["wm1bT"][:],
                                 rhs=h[:, pm * HC_:(pm + 1) * HC_],
                                 start=True, stop=True)
            if g % 2 == 1:
                nc.vector.tensor_reduce(
                    out=ap_of(Hfin, (g - 1) * 60,
                              [list(Hfin.ap[0]), [30, 4], [1, 30]]),
                    in_=ap_of(ps2big, 0,
                              [list(ps2big.ap[0]), [512, 4], [15, 30], [1, 15]]),
                    axis=AX.X, op=OP.max)

        # ---- head ----
        Hb = big.tile([HID, COLS], f32r, tag="bigA")  # reuse init1 slot
        HC = big.tile([HID, P * N], f32r)
        for t in range(8):
            sl = slice(t * 480, (t + 1) * 480)
            nc.scalar.activation(Hb[:, sl], Hfin[:, sl], AF.Relu,
                                 bias=sb["bm1b"][:])
        for t in range(8):
            q = t // 4
            sl = slice(t * 480, (t + 1) * 480)
            ps = ps_1.tile([HID, 1024], f32, tag="ps1", name=f"psH_{t}")
            nc.tensor.matmul(ps[:, 0:480], lhsT=sb["wcaT1" if q == 0 else "wcaT2"][:],
                             rhs=Hb[:, sl], start=True, stop=True)
            nc.scalar.activation(HC[:, sl], ps[:, 0:480], AF.Relu,
                                 bias=sb["bca1" if q == 0 else "bca2"][:])
        qrow = big.tile([1, P * N], f32)
        for t in range(8):
            q = t // 4
            sl = slice(t * 480, (t + 1) * 480)
            ps = ps_1.tile([HID, 1024], f32, tag="ps1", name=f"psQ_{t}")
            nc.tensor.matmul(ps[0:1, 0:480], lhsT=sb["wcb1" if q == 0 else "wcb2"][:],
                             rhs=HC[:, sl], start=True, stop=True)
            nc.vector.tensor_copy(qrow[:, sl], ps[0:1, 0:480])
        nc.sync.dma_start(out=qout.ap(), in_=qrow[:])

    nc.compile()
    split_excess_waits(nc)
    return nc


_CACHED = {}


def _get_program():
    if "nc" not in _CACHED:
        _CACHED["nc"] = build_program()
    return _CACHED["nc"]


def _split_bf16(a):
    hi = a.astype(ml_dtypes.bfloat16)
    lo = (a - hi.astype(np.float32)).astype(ml_dtypes.bfloat16)
    return hi, lo


def _host_inputs(state, action, w):
    nodes1 = np.concatenate(
        [state.reshape(BS, N, 2), action.reshape(BS, N, 2)], axis=-1)
    nodes2 = np.concatenate([state, action], axis=1).reshape(BS, N, 4)

    Wa = w["W_m1a"]                     # (128, 388)
    Wa1, Wa2 = Wa[:, :194], Wa[:, 194:]
    Wau = Wa1 - Wa2
    wa2aT = np.ascontiguousarray(Wa2[:, :128].T)   # (128, 128) f_in x f_out
    wauT = np.ascontiguousarray(Wau[:, :128].T)
    wa2aT_hi, wa2aT_lo = _split_bf16(wa2aT)
    wauT_hi, wauT_lo = _split_bf16(wauT)

    # cls-const contributions: row r = q*3 + cat
    c6v = np.zeros((6, HID), np.float32)
    c6u = np.zeros((6, HID), np.float32)
    for q in range(2):
        emb = w["emb1"] if q == 0 else w["emb2"]   # (3, 64)
        cls = np.maximum(emb, 0.0)
        c6v[q * 3:q * 3 + 3] = cls @ Wa2[:, 128:192].T
        c6u[q * 3:q * 3 + 3] = cls @ Wau[:, 128:192].T

    # ind6[r, col]: col = p*32+i -> q = p//64, cat = i//10 (dead cols: 0)
    ind6 = np.zeros((6, COLS), np.float32)
    pidx = np.arange(P)
    for p in range(P):
        q = p // 64
        for cat in range(3):
            ind6[q * 3 + cat, p * ST + cat * 10: p * ST + (cat + 1) * 10] = 1.0

    # R_U const one-hot [64, 900]: col c = pm*450 + i*15 + s -> row 32*pm + i
    r_u = np.zeros((64, GC), np.float32)
    for pm in range(2):
        for i in range(N):
            r_u[ST * pm + i, pm * HC_ + i * 15:pm * HC_ + (i + 1) * 15] = 1.0

    def dup16(W):  # (128, 4) -> [Whi(4); Whi(4); Wlo(4); Wlo(4)].T stacked
        hi, lo = _split_bf16(np.ascontiguousarray(W.T))   # (4, 128) each
        return np.concatenate([hi, hi, lo, lo], 0)        # (16, 128)


    shared = {
        "wiT1d": dup16(w["W_init1"]), "wiT2d": dup16(w["W_init2"]),
        "wa2aT_hi": wa2aT_hi, "wa2aT_lo": wa2aT_lo,
        "wauT_hi": wauT_hi, "wauT_lo": wauT_lo,
        "c6v": c6v.astype(ml_dtypes.bfloat16), "c6u": c6u.astype(ml_dtypes.bfloat16),
        "ind6": ind6.astype(ml_dtypes.bfloat16),
        "wm1bT": np.ascontiguousarray(w["W_m1b"].T),
        "wcaT1": np.ascontiguousarray(w["W_c1a"].T),
        "wcaT2": np.ascontiguousarray(w["W_c2a"].T),
        "wcb1": np.ascontiguousarray(w["W_c1b"].T),
        "wcb2": np.ascontiguousarray(w["W_c2b"].T),
        "bi1": w["b_init1"].reshape(HID, 1), "bi2": w["b_init2"].reshape(HID, 1),
        "bm1a": w["b_m1a"].reshape(HID, 1), "bm1b": w["b_m1b"].reshape(HID, 1),
        "bca1": w["b_c1a"].reshape(HID, 1), "bca2": w["b_c2a"].reshape(HID, 1),
        "r_u": r_u.astype(ml_dtypes.bfloat16),
        "iota64": np.arange(64, dtype=np.float32).reshape(64, 1),
        "p32": ((np.arange(P) % 2) * ST).astype(np.float32).reshape(P, 1),
    }
    shared = {k: np.ascontiguousarray(v) for k, v in shared.items()}

    in_maps = []
    for c in range(CORES):
        x4 = np.concatenate(
            [nodes1[c * BSC:(c + 1) * BSC], nodes2[c * BSC:(c + 1) * BSC]], axis=0)
        # x_T padded to stride 32: [4, 4096]
        xp = np.zeros((P, ST, 4), np.float32)
        xp[:, :N, :] = x4
        x_T = np.ascontiguousarray(xp.reshape(COLS, 4).T)
        x_hi, x_lo = _split_bf16(x_T)
        m = dict(shared)
        m["xst"] = np.ascontiguousarray(
            np.concatenate([x_hi, x_lo, x_hi, x_lo], 0))
        m["x_pp"] = np.ascontiguousarray(x4.reshape(P, N * 4))
        in_maps.append(m)
    return in_maps


def kernel(**inputs):
    state = np.asarray(inputs["state"], np.float32)
    action = np.asarray(inputs["action"], np.float32)
    weights = {k: np.asarray(v, np.float32) for k, v in inputs.items()
               if k not in ("state", "action")}
    nc = _get_program()
    in_maps = _host_inputs(state, action, weights)
    bcb1 = float(weights["b_c1b"].reshape(-1)[0])
    bcb2 = float(weights["b_c2b"].reshape(-1)[0])
    for attempt in range(3):
        res = run_bass_kernel_spmd(nc, in_maps, core_ids=list(range(CORES)))
        q1 = np.zeros((BS, N), np.float32)
        q2 = np.zeros((BS, N), np.float32)
        for c in range(CORES):
            arr = res.results[c]["qout"].reshape(P, N)   # col = p*30 + i
            q1[c * BSC:(c + 1) * BSC] = arr[:BSC] + bcb1
            q2[c * BSC:(c + 1) * BSC] = arr[BSC:] + bcb2
        # guard against cold-start device flakes (uninitialized reads show
        # up as huge values); outputs are O(0.05) for any sane input scale
        m = max(np.abs(q1).max(), np.abs(q2).max())
        if np.isfinite(m) and m < 1e4:
            break
    return (q1, q2)


if __name__ == "__main__":
    print("smoke build only")
    build_program()
    print("built ok")


# revision 6
# speedup vs baseline: 1.0961x; 1.0961x over previous
"""Trainium2 Bass kernel for nn_CriticOld (twin-Q GNN critic: per-sample kNN +
EdgeConv + MLP head), data-parallel over batch across 8 NeuronCores.

V3 layout (per core): 128 problems = 2 Q-nets x 64 samples; col = p*32 + i
(i<30 real, 2 dead cols per problem), COLS = 4096. EdgeConv pairs are built
ON THE PE as psum1 = V_sb.T @ R_V + U_sb.T @ R_U where R_V is a runtime
one-hot (built by a 4x-packed DVE is_equal against broadcast index codes)
and R_U a constant one-hot. This kills the HBM V-table, the 61k-descriptor
dma_gather and the identity-recombine matmuls of the previous version.
kNN distances run on the Pool engine, top-15 on DVE, relu-evac on Act,
slot-max on DVE over dual-bank psum tiles.
"""
import sys

sys.path.insert(0, "/opt/trn_rl_repo")

import numpy as np
import ml_dtypes

import concourse.bass as bass
from concourse import bacc
import concourse.mybir as mybir
import concourse.tile as tile
from concourse import library_config
from concourse.bass_utils import run_bass_kernel_spmd
from concourse.vector_clock import ScopedClock

f32 = mybir.dt.float32
f32r = mybir.dt.float32r
bf16 = mybir.dt.bfloat16
u16 = mybir.dt.uint16
i16 = mybir.dt.int16
AF = mybir.ActivationFunctionType
OP = mybir.AluOpType
AX = mybir.AxisListType

BS, N, K, HID, EMB = 512, 30, 15, 128, 64
CORES = 8
BSC = BS // CORES          # samples per core
P = 2 * BSC                # 128 problems per core (2 Q-nets)
ST = 32                    # col stride per problem (30 real + 2 dead)
COLS = P * ST              # 4096
NG = 64                    # phase-D groups (2 problems each)
GC = 2 * K * N             # 900 pair cols per group
HC_ = K * N                # 450 pair cols per problem

import os
KNOB_POOL_CMP = int(os.environ.get("KPOOLCMP", "0"))  # every n-th group's compare on Pool
KNOB_POOL_DIST = os.environ.get("KPOOLDIST", "0") == "1"


def _patched_drain_and_barrier(self, tick_clock, wait_clock):
    # this walrus build caps sync-waits at 1/instruction; spread the
    # end-of-kernel waits over SP NOPs instead of one multi-wait Drain.
    nc = self.nc
    probe = nc.sync.nop()
    wait_clock.add_sem_waits(probe.ins, ScopedClock({None: tick_clock.global_clock}))
    si = probe.ins.sync_info
    waits = list(si.on_wait) if si is not None else []
    if len(waits) > 1:
        si.on_wait = [waits[0]]
        for w in waits[1:]:
            extra = nc.sync.nop()
            esi = extra.ins.sync_info
            if esi is None:
                extra.ins.sync_info = mybir.SyncInfo(on_wait=[w], on_update=[])
            else:
                esi.on_wait = [w]
    nc.sync.drain()
    nc.all_engine_barrier()
    assert self.sems is not None
    popped = nc._tile_sem_poison_stack.pop()
    assert popped is self._sem_poison
    nc.clear_and_free_semaphores(list(self.sems.allocated().values()))
    nc.all_engine_barrier()


tile.TileContext._drain_and_barrier = _patched_drain_and_barrier

_ws_cnt = [0]


def split_excess_waits(nc):
    """This walrus build supports at most 1 sync-wait per instruction (2 for
    EventSemaphore). Tile attaches several; move the extras onto same-engine
    NOPs inserted right before the instruction."""
    for fn in nc.m.functions:
        for bb in fn.blocks:
            new_list = []
            for inst in bb.instructions:
                si = inst.sync_info
                cap = 2 if isinstance(inst, mybir.InstEventSemaphore) else 1
                if si is not None and si.on_wait is not None and len(si.on_wait) > cap:
                    waits = list(si.on_wait)
                    for w in waits[:-cap]:
                        n = mybir.InstNoOp(name=f"I-wsplit-{_ws_cnt[0]}", ins=[], outs=[])
                        _ws_cnt[0] += 1
                        n.engine = inst.engine
                        n.sync_info = mybir.SyncInfo(on_wait=[w], on_update=[])
                        nc.register_instruction(n, overwrite=True)
                        new_list.append(n)
                    si.on_wait = waits[-cap:]
                new_list.append(inst)
            bb.instructions[:] = new_list


def ap_of(t, offset, dims):
    """Build a raw AP on tile/dram tensor t: dims = [[step, count], ...] (elements)."""
    base = t.ap() if hasattr(t, "ap") and not isinstance(t, bass.AP) else t
    return bass.AP(tensor=base.tensor, offset=base.offset + offset, ap=dims)


def build_program():
    nc = bacc.Bacc("TRN2", debug=False)

    din = {}
    def inp(name, shape, dtype=f32):
        din[name] = nc.dram_tensor(name, shape, dtype, kind="ExternalInput")
        return din[name]

    inp("xst", [16, COLS], bf16)          # [x_hi; x_lo; x_hi; x_lo]
    inp("x_pp", [P, N * 4])
    inp("wiT1d", [16, HID], bf16); inp("wiT2d", [16, HID], bf16)
    inp("wa2aT_hi", [HID, HID], bf16); inp("wa2aT_lo", [HID, HID], bf16)
    inp("wauT_hi", [HID, HID], bf16); inp("wauT_lo", [HID, HID], bf16)
    inp("c6v", [6, HID], bf16); inp("c6u", [6, HID], bf16)
    inp("ind6", [6, COLS], bf16)
    inp("wm1bT", [HID, HID], f32)
    inp("wcaT1", [HID, HID]); inp("wcaT2", [HID, HID])
    inp("wcb1", [HID, 1]); inp("wcb2", [HID, 1])
    inp("bi1", [HID, 1]); inp("bi2", [HID, 1])
    inp("bm1a", [HID, 1]); inp("bm1b", [HID, 1])
    inp("bca1", [HID, 1]); inp("bca2", [HID, 1])
    inp("r_u", [64, GC], bf16)
    inp("iota64", [64, 1])
    inp("p32", [P, 1])

    idxc_dram = nc.dram_tensor("idxc_dram", [P, HC_], i16)  # internal
    qout = nc.dram_tensor("qout", [1, P * N], f32, kind="ExternalOutput")

    from contextlib import ExitStack
    ctx = ExitStack()
    with tile.TileContext(nc) as tc, ctx:
        consts = ctx.enter_context(tc.tile_pool(name="consts", bufs=1))
        big = ctx.enter_context(tc.tile_pool(name="big", bufs=1))
        knn = ctx.enter_context(tc.tile_pool(name="knn", bufs=1))
        t8p = ctx.enter_context(tc.tile_pool(name="t8p", bufs=8))
        bcp = ctx.enter_context(tc.tile_pool(name="bcp", bufs=2))
        rpool = ctx.enter_context(tc.tile_pool(name="rpool", bufs=1))
        hpool = ctx.enter_context(tc.tile_pool(name="hpool", bufs=4))
        ps_1 = ctx.enter_context(tc.tile_pool(name="ps_1", bufs=2, space="PSUM"))
        ps_2 = ctx.enter_context(tc.tile_pool(name="ps_2", bufs=2, space="PSUM"))

        if KNOB_POOL_DIST or KNOB_POOL_CMP:
            nc.gpsimd.load_library(library_config.mlp)

        # ---- load inputs to SBUF ----
        sb = {}
        F32R_NAMES = ("wm1bT", "wcaT1", "wcaT2", "wcb1", "wcb2")
        for name in din:
            t = din[name]
            dt = f32r if name in F32R_NAMES else t.dtype
            st_ = consts.tile([t.shape[0], t.shape[1]], dt, tag=name)
            src_ap = t.ap().bitcast(f32r) if name in F32R_NAMES else t.ap()
            nc.sync.dma_start(out=st_[:], in_=src_ap)
            sb[name] = st_

        # ---- kNN distances (Pool) ----
        xpp = sb["x_pp"]
        diff = knn.tile([P, 3600], f32)
        sqd = knn.tile([P, 3600], f32)
        negd = knn.tile([P, 900], f32)
        nc.vector.tensor_tensor(
            out=diff[:],
            in0=ap_of(xpp, 0, [list(xpp.ap[0]), [4, N], [0, N], [1, 4]]),
            in1=ap_of(xpp, 0, [list(xpp.ap[0]), [0, N], [4, N], [1, 4]]),
            op=OP.subtract)
        nc.scalar.activation(sqd[:], diff[:], AF.Square)
        # negd = -sum(diff^2); self-distance 0 is always rank 1 of the top-k,
        # so the diagonal needs no exclusion mask (skipped positionally below)
        nc.vector.tensor_reduce(
            out=negd[:], in_=ap_of(sqd, 0, [list(sqd.ap[0]), [4, 900], [1, 4]]),
            axis=AX.X, op=OP.add, negate=True)

        # ---- top-15 per node (DVE) ----
        idxall = knn.tile([P, 16 * N], u16)
        for i in range(N):
            nd = negd[:, i * 30:(i + 1) * 30]
            m8 = t8p.tile([P, 8], f32, tag="m8")
            scr = t8p.tile([P, 30], f32, tag="scr")
            m8b = t8p.tile([P, 8], f32, tag="m8b")
            nc.vector.max(m8[:], nd)
            nc.vector.max_index(idxall[:, i * 16:i * 16 + 8], m8[:], nd)
            nc.vector.match_replace(scr[:], in_to_replace=m8[:], in_values=nd,
                                    imm_value=-1e30)
            nc.vector.max(m8b[:], scr[:])
            nc.vector.max_index(idxall[:, i * 16 + 8:i * 16 + 16], m8b[:], scr[:])

        # idxc_P = idxall (c-order i*15+s) + 32*(p%2), as i16 in DRAM
        gfp = knn.tile([P, HC_], f32)
        # slot 0 of round-1 max8 is the self-match (distance 0): skip it
        nc.vector.tensor_scalar(
            out=ap_of(gfp, 0, [list(gfp.ap[0]), [15, N], [1, 7]]),
            in0=ap_of(idxall, 1, [list(idxall.ap[0]), [16, N], [1, 7]]),
            scalar1=sb["p32"][:], scalar2=None, op0=OP.add)
        nc.vector.tensor_scalar(
            out=ap_of(gfp, 7, [list(gfp.ap[0]), [15, N], [1, 8]]),
            in0=ap_of(idxall, 8, [list(idxall.ap[0]), [16, N], [1, 8]]),
            scalar1=sb["p32"][:], scalar2=None, op0=OP.add)
        gi16 = knn.tile([P, HC_], i16)
        nc.vector.tensor_copy(gi16[:], gfp[:])
        nc.sync.dma_start(out=idxc_dram.ap(), in_=gi16[:])

        # broadcast index codes: 8 chunks x [64 part, 8 groups * 900] i16
        bch = []
        for c in range(8):
            t = bcp.tile([64, 8 * GC], i16, tag="bc", name=f"bc_{c}")
            src = bass.AP(tensor=idxc_dram, offset=(16 * c) * HC_,
                          ap=[[0, 64], [2 * HC_, 8], [HC_, 2], [1, HC_]])
            nc.sync.dma_start(out=t[:], in_=src)
            bch.append(t)

        # ---- phase A: init1 = relu(x W_i^T + b_i), bf16 feature-major ----
        init1 = big.tile([HID, COLS], bf16, tag="bigA")
        for t in range(8):
            q = t // 4
            sl = slice(t * 512, (t + 1) * 512)
            ps = ps_1.tile([HID, 1024], f32, tag="ps1", name=f"psA_{t}")
            nc.tensor.matmul(ps[:, 0:512], lhsT=sb["wiT1d" if q == 0 else "wiT2d"][:],
                             rhs=sb["xst"][:, sl], start=True, stop=True)
            nc.scalar.activation(init1[:, sl], ps[:, 0:512], AF.Relu,
                                 bias=sb["bi1" if q == 0 else "bi2"][:])

        # ---- VU_sb: node-major values, V at partitions 0:64, U at 64:128 ----
        # VU_sb[:, blk*128+f]: partition p<64 = V[col=blk*64+p, f],
        #                      partition 64+p = U[col=blk*64+p, f].
        VU_sb = big.tile([HID, 64 * HID], bf16)
        for t8 in range(8):  # 8 x 64-col blocks per psum tile
            ps = ps_2.tile([HID, 1024], f32, tag="ps2", name=f"psVU_{t8}")
            for k in range(8):
                blk = t8 * 8 + k
                lhs = init1[:, blk * 64:(blk + 1) * 64]
                ind = sb["ind6"][:, blk * 64:(blk + 1) * 64]
                seg = slice(k * 128, k * 128 + 128)
                nc.tensor.matmul(ps[0:64, seg], lhsT=lhs, rhs=sb["wa2aT_hi"][:],
                                 start=True, stop=False)
                nc.tensor.matmul(ps[0:64, seg], lhsT=lhs, rhs=sb["wa2aT_lo"][:],
                                 start=False, stop=False)
                nc.tensor.matmul(ps[0:64, seg], lhsT=ind, rhs=sb["c6v"][:],
                                 start=False, stop=True)
                nc.tensor.matmul(ps[64:128, seg], lhsT=lhs, rhs=sb["wauT_hi"][:],
                                 start=True, stop=False)
                nc.tensor.matmul(ps[64:128, seg], lhsT=lhs, rhs=sb["wauT_lo"][:],
                                 start=False, stop=False)
                nc.tensor.matmul(ps[64:128, seg], lhsT=ind, rhs=sb["c6u"][:],
                                 start=False, stop=True)
            nc.scalar.activation(VU_sb[:, t8 * 1024:(t8 + 1) * 1024], ps[:],
                                 AF.Copy)

        # ---- phase D: 64 groups of 2 problems / 900 pair-cols ----
        # R buffers: rows 64:128 hold the constant U one-hots (loaded once
        # straight from the r_u input); compare rewrites rows 0:64 per group.
        NRV = 6
        rvb = []
        for i in range(NRV):
            t = rpool.tile([HID, GC], bf16, tag=f"rv{i}")
            nc.sync.dma_start(out=t[64:128, :], in_=din["r_u"].ap())
            rvb.append(t)
        Hfin = big.tile([HID, P * N], f32)
        for g in range(NG):
            rv = rvb[g % NRV]
            nc.vector.tensor_scalar(out=rv[0:64, :],
                               in0=bch[g // 8][:, (g % 8) * GC:(g % 8 + 1) * GC],
                               scalar1=sb["iota64"][:], scalar2=None,
                               op0=OP.is_equal)
            ps1 = ps_1.tile([HID, 1024], f32, tag="ps1", name=f"ps1_{g}")
            for pm in range(2):
                nc.tensor.matmul(ps1[:, pm * 512:pm * 512 + HC_],
                                 lhsT=VU_sb[:, g * 128:(g + 1) * 128],
                                 rhs=rv[:, pm * HC_:(pm + 1) * HC_],
                                 start=True, stop=True)
            h = hpool.tile([HID, GC], f32r, tag="h")
            nc.scalar.activation(
                ap_of(h, 0, [list(h.ap[0]), [HC_, 2], [1, HC_]]),
                ap_of(ps1, 0, [list(ps1.ap[0]), [512, 2], [1, HC_]]),
                AF.Relu, bias=sb["bm1a"][:])
            ps2 = ps_2.tile([HID, 1024], f32, tag="ps2", name=f"ps2_{g}")
            for pm in range(2):
                nc.tensor.matmul(ps2[:, pm * 512:pm * 512 + HC_],
                                 lhsT=sb["wm1bT"][:],
                                 rhs=h[:, pm * HC_:(pm + 1) * HC_],
                                 start=True, stop=True)
            nc.vector.tensor_reduce(
                out=ap_of(Hfin, g * 60, [list(Hfin.ap[0]), [30, 2], [1, 30]]),
                in_=ap_of(ps2, 0, [list(ps2.ap[0]), [512, 2], [15, 30], [1, 15]]),
                axis=AX.X, op=OP.max)

        # ---- head ----
        Hb = big.tile([HID, COLS], f32r, tag="bigA")  # reuse init1 slot
        HC = big.tile([HID, P * N], f32r)
        for t in range(8):
            sl = slice(t * 480, (t + 1) * 480)
            nc.scalar.activation(Hb[:, sl], Hfin[:, sl], AF.Relu,
                                 bias=sb["bm1b"][:])
        for t in range(8):
            q = t // 4
            sl = slice(t * 480, (t + 1) * 480)
            ps = ps_1.tile([HID, 1024], f32, tag="ps1", name=f"psH_{t}")
            nc.tensor.matmul(ps[:, 0:480], lhsT=sb["wcaT1" if q == 0 else "wcaT2"][:],
                             rhs=Hb[:, sl], start=True, stop=True)
            nc.scalar.activation(HC[:, sl], ps[:, 0:480], AF.Relu,
                                 bias=sb["bca1" if q == 0 else "bca2"][:])
        qrow = big.tile([1, P * N], f32)
        for t in range(8):
            q = t // 4
            sl = slice(t * 480, (t + 1) * 480)
            ps = ps_2.tile([HID, 1024], f32, tag="ps2", name=f"psQ_{t}")
            nc.tensor.matmul(ps[0:1, 0:480], lhsT=sb["wcb1" if q == 0 else "wcb2"][:],
                             rhs=HC[:, sl], start=True, stop=True)
            nc.vector.tensor_copy(qrow[:, sl], ps[0:1, 0:480])
        nc.sync.dma_start(out=qout.ap(), in_=qrow[:])

    nc.compile()
    split_excess_waits(nc)
    return nc


_CACHED = {}


def _get_program():
    if "nc" not in _CACHED:
        _CACHED["nc"] = build_program()
    return _CACHED["nc"]


def _split_bf16(a):
    hi = a.astype(ml_dtypes.bfloat16)
    lo = (a - hi.astype(np.float32)).astype(ml_dtypes.bfloat16)
    return hi, lo


def _host_inputs(state, action, w):
    nodes1 = np.concatenate(
        [state.reshape(BS, N, 2), action.reshape(BS, N, 2)], axis=-1)
    nodes2 = np.concatenate([state, action], axis=1).reshape(BS, N, 4)

    Wa = w["W_m1a"]                     # (128, 388)
    Wa1, Wa2 = Wa[:, :194], Wa[:, 194:]
    Wau = Wa1 - Wa2
    wa2aT = np.ascontiguousarray(Wa2[:, :128].T)   # (128, 128) f_in x f_out
    wauT = np.ascontiguousarray(Wau[:, :128].T)
    wa2aT_hi, wa2aT_lo = _split_bf16(wa2aT)
    wauT_hi, wauT_lo = _split_bf16(wauT)

    # cls-const contributions: row r = q*3 + cat
    c6v = np.zeros((6, HID), np.float32)
    c6u = np.zeros((6, HID), np.float32)
    for q in range(2):
        emb = w["emb1"] if q == 0 else w["emb2"]   # (3, 64)
        cls = np.maximum(emb, 0.0)
        c6v[q * 3:q * 3 + 3] = cls @ Wa2[:, 128:192].T
        c6u[q * 3:q * 3 + 3] = cls @ Wau[:, 128:192].T

    # ind6[r, col]: col = p*32+i -> q = p//64, cat = i//10 (dead cols: 0)
    ind6 = np.zeros((6, COLS), np.float32)
    pidx = np.arange(P)
    for p in range(P):
        q = p // 64
        for cat in range(3):
            ind6[q * 3 + cat, p * ST + cat * 10: p * ST + (cat + 1) * 10] = 1.0

    # R_U const one-hot [64, 900]: col c = pm*450 + i*15 + s -> row 32*pm + i
    r_u = np.zeros((64, GC), np.float32)
    for pm in range(2):
        for i in range(N):
            r_u[ST * pm + i, pm * HC_ + i * 15:pm * HC_ + (i + 1) * 15] = 1.0

    def dup16(W):  # (128, 4) -> [Whi(4); Whi(4); Wlo(4); Wlo(4)].T stacked
        hi, lo = _split_bf16(np.ascontiguousarray(W.T))   # (4, 128) each
        return np.concatenate([hi, hi, lo, lo], 0)        # (16, 128)


    shared = {
        "wiT1d": dup16(w["W_init1"]), "wiT2d": dup16(w["W_init2"]),
        "wa2aT_hi": wa2aT_hi, "wa2aT_lo": wa2aT_lo,
        "wauT_hi": wauT_hi, "wauT_lo": wauT_lo,
        "c6v": c6v.astype(ml_dtypes.bfloat16), "c6u": c6u.astype(ml_dtypes.bfloat16),
        "ind6": ind6.astype(ml_dtypes.bfloat16),
        "wm1bT": np.ascontiguousarray(w["W_m1b"].T),
        "wcaT1": np.ascontiguousarray(w["W_c1a"].T),
        "wcaT2": np.ascontiguousarray(w["W_c2a"].T),
        "wcb1": np.ascontiguousarray(w["W_c1b"].T),
        "wcb2": np.ascontiguousarray(w["W_c2b"].T),
        "bi1": w["b_init1"].reshape(HID, 1), "bi2": w["b_init2"].reshape(HID, 1),
        "bm1a": w["b_m1a"].reshape(HID, 1), "bm1b": w["b_m1b"].reshape(HID, 1),
        "bca1": w["b_c1a"].reshape(HID, 1), "bca2": w["b_c2a"].reshape(HID, 1),
        "r_u": r_u.astype(ml_dtypes.bfloat16),
        "iota64": np.arange(64, dtype=np.float32).reshape(64, 1),
        "p32": ((np.arange(P) % 2) * ST).astype(np.float32).reshape(P, 1),
    }
    shared = {k: np.ascontiguousarray(v) for k, v in shared.items()}

    in_maps = []
    for c in range(CORES):
        x4 = np.concatenate(
            [nodes1[c * BSC:(c + 1) * BSC], nodes2[c * BSC:(c + 1) * BSC]], axis=0)
        # x_T padded to stride 32: [4, 4096]
        xp = np.zeros((P, ST, 4), np.float32)
        xp[:, :N, :] = x4
        x_T = np.ascontiguousarray(xp.reshape(COLS, 4).T)
        x_hi, x_lo = _split_bf16(x_T)
        m = dict(shared)
        m["xst"] = np.ascontiguousarray(
            np.concatenate([x_hi, x_lo, x_hi, x_lo], 0))
        m["x_pp"] = np.ascontiguousarray(x4.reshape(P, N * 4))
        in_maps.append(m)
    return in_maps


def kernel(**inputs):
    state = np.asarray(inputs["state"], np.float32)
    action = np.asarray(inputs["action"], np.float32)
    weights = {k: np.asarray(v, np.float32) for k, v in inputs.items()
               if k not in ("state", "action")}
    nc = _get_program()
    in_maps = _host_inputs(state, action, weights)
    bcb1 = float(weights["b_c1b"].reshape(-1)[0])
    bcb2 = float(weights["b_c2b"].reshape(-1)[0])
    for attempt in range(3):
        res = run_bass_kernel_spmd(nc, in_maps, core_ids=list(range(CORES)))
        q1 = np.zeros((BS, N), np.float32)
        q2 = np.zeros((BS, N), np.float32)
        for c in range(CORES):
            arr = res.results[c]["qout"].reshape(P, N)   # col = p*30 + i
            q1[c * BSC:(c + 1) * BSC] = arr[:BSC] + bcb1
            q2[c * BSC:(c + 1) * BSC] = arr[BSC:] + bcb2
        # guard against cold-start device flakes (uninitialized reads show
        # up as huge values); outputs are O(0.05) for any sane input scale
        m = max(np.abs(q1).max(), np.abs(q2).max())
        if np.isfinite(m) and m < 1e4:
            break
    return (q1, q2)


if __name__ == "__main__":
    print("smoke build only")
    build_program()
    print("built ok")
